# revision 1
# baseline (speedup 1.0000x reference)
"""GraphSAGE layer (mean-aggr SAGEConv + BatchNorm1d) on 8 Trainium2 NeuronCores.

Strategy (edge-cut partitioning by destination node):
  - Nodes are split into 8 equal ranges (12500/core); each core owns all edges
    whose dst falls in its range, so aggregation completes locally.
  - Host groups edges by (core, dst-block of 128), pads each group to a
    multiple of 128 and emits, per edge slot: the int32 source index (for the
    indirect gather DMA), the f32 local dst slot (0..127, -1 for padding) and
    the f32 edge weight w = 1/max(deg[dst],1) (0 for padding).
  - On device, per 128-node block:
      * indirect_dma_start gathers x[src] rows (fp16) for the block's edges,
      * a one-hot "selection" matrix S'[e,d] = (dstloc[e]==d) * w[e] is built
        with a single DVE tensor_scalar, and PE computes
        aggT[f,d] += G[e,f]^T @ S'[e,d] accumulating in PSUM -> mean aggregate.
      * x_rawT[j,d] = W_l^T @ aggT + W_r^T @ xT + b_l (PE, PSUM accumulate)
      * BatchNorm stats (sum, sum of squares per feature) come free via the
        scalar engine's accum_out while copying PSUM->SBUF.
  - BN statistics are AllReduced across the 8 cores (DRAM collective), then a
    second pass applies y = x_raw * scale + shift per feature (features live on
    partitions, so this is a per-partition DVE tensor_scalar).
  - Outputs are written feature-major ([128, nodes]) and transposed on host.
"""

import os
from dataclasses import dataclass

import numpy as np

# concourse ships with the container; it is an installed package, not a sibling file.
import concourse.bacc as bacc
import concourse.bass as bass
import concourse.mybir as mybir
import concourse.tile as tile
from concourse.bass_utils import run_bass_kernel_spmd

F16 = mybir.dt.float16
F32 = mybir.dt.float32
I32 = mybir.dt.int32
ALU = mybir.AluOpType
ACT = mybir.ActivationFunctionType

D = 128
P = 128

LAST_EXEC_NS = None  # filled by run_graph when trace=True


@dataclass
class Cfg:
    N: int
    ncores: int = 8
    sb: int = 7  # dst blocks per superblock (pipeline/staging unit)

    @property
    def npc(self):  # nodes per core
        assert self.N % self.ncores == 0
        return self.N // self.ncores

    @property
    def nblk(self):  # 128-node dst blocks per core
        return (self.npc + P - 1) // P

    @property
    def last_valid(self):  # valid nodes in the final block
        return self.npc - (self.nblk - 1) * P

    @property
    def sblocks(self):  # list of block ranges, one per superblock
        out = []
        b = 0
        while b < self.nblk:
            out.append(list(range(b, min(b + self.sb, self.nblk))))
            b += self.sb
        return out


def _layout(cfg, NT):
    """Column layout. NT[b] = #128-edge tiles for dst-block b (shared across
    cores). Columns are ordered superblock-major then block then tile, so each
    superblock's gather is one contiguous range."""
    colbase = np.zeros(cfg.nblk, dtype=np.int64)
    sbinfo = []
    col = 0
    for blocks in cfg.sblocks:
        sb_c0 = col
        for b in blocks:
            colbase[b] = col
            col += int(NT[b])
        sbinfo.append((sb_c0, col - sb_c0))
    return int(col), colbase, sbinfo


def preprocess(cfg, x, edge_index, W_l, b_l, W_r, gamma, beta):
    """Host-side sharding: group edges by (core, block), compute the shared
    tile-count table NT, and emit per-core device arrays."""
    N, npc, nblk = cfg.N, cfg.npc, cfg.nblk
    src = np.asarray(edge_index[0], dtype=np.int64)
    dst = np.asarray(edge_index[1], dtype=np.int64)
    E = src.shape[0]

    deg = np.bincount(dst, minlength=N)
    w_node = (1.0 / np.maximum(deg, 1.0)).astype(np.float32)

    core = dst // npc
    noderel = dst - core * npc
    blk = noderel >> 7
    dloc = noderel & 127

    key = core * nblk + blk
    src_core0 = src // npc
    is_remote = (src_core0 != core).astype(np.int64)
    order = np.argsort(key * 2 + is_remote, kind="stable")
    ks = key[order]
    cnt = np.bincount(key, minlength=cfg.ncores * nblk).reshape(cfg.ncores, nblk)
    NT = (cnt.max(axis=0) + 127) // 128  # [nblk]
    loc_cnt = np.bincount(key[is_remote == 0],
                          minlength=cfg.ncores * nblk).reshape(cfg.ncores, nblk)
    # tiles guaranteed all-local on EVERY core: they read the core's own x
    # slice directly and do not wait for the x AllGather
    NTloc = np.minimum((loc_cnt // 128).min(axis=0), NT)

    total_cols, colbase, sbinfo = _layout(cfg, NT)
    slots = total_cols * P
    slot_base = colbase * P

    # rank of each edge within its (core, blk) group
    grp_first = np.r_[0, np.flatnonzero(np.diff(ks)) + 1]
    starts = np.zeros(E, dtype=np.int64)
    starts[grp_first] = grp_first
    starts = np.maximum.accumulate(starts)
    rank = np.arange(E, dtype=np.int64) - starts

    # gather-table row ids in the padded, core-concatenated table layout
    src_core = src_core0
    src_loc = (src - src_core * npc).astype(np.int32)
    src_pad = (src_core * (nblk * P) + src_loc).astype(np.int32)

    per_core = []
    bounds = np.searchsorted(ks, np.arange(cfg.ncores + 1) * nblk)
    for c in range(cfg.ncores):
        a, b = bounds[c], bounds[c + 1]
        ecs = order[a:b]
        slot = slot_base[ks[a:b] - c * nblk] + rank[a:b]

        a_src = np.zeros(slots, dtype=np.int32)
        a_dl = np.full(slots, -1.0, dtype=np.float16)
        a_w = np.zeros(slots, dtype=np.float16)
        ksb = ks[a:b] - c * nblk
        tloc = (slot - slot_base[ksb]) >> 7
        a_src[slot] = np.where(tloc < NTloc[ksb], src_loc[ecs], src_pad[ecs])
        a_dl[slot] = dloc[ecs].astype(np.float16)
        a_w[slot] = w_node[dst[ecs]].astype(np.float16)

        # edge slot s lives at [partition s%128, column s//128]
        idx_t = np.ascontiguousarray(a_src.reshape(-1, P).T)
        dl_t = np.ascontiguousarray(a_dl.reshape(-1, P).T)
        w_t = np.ascontiguousarray(a_w.reshape(-1, P).T)

        x16s = np.zeros((nblk * P, D), dtype=np.float16)
        x16s[:npc] = np.asarray(x[c * npc:(c + 1) * npc], dtype=np.float16)

        per_core.append(dict(idx=idx_t, dl=dl_t, wv=w_t, x16s=x16s))

    shared = dict(
        wl=np.asarray(W_l, dtype=np.float16),
        wr=np.asarray(W_r, dtype=np.float16),
        blr=np.asarray(b_l, dtype=np.float16).reshape(1, D),
        gamma=np.asarray(gamma, dtype=np.float32).reshape(P, 1),
        beta=np.asarray(beta, dtype=np.float32).reshape(P, 1),
        iota=np.tile(np.arange(P, dtype=np.float16), (P, 1)),
    )
    return (NT, NTloc), per_core, shared


def build_program(cfg, NTs):
    NT, NTloc = NTs
    total_cols, colbase, sbinfo = _layout(cfg, NT)
    N, nblk, npc = cfg.N, cfg.nblk, cfg.npc

    nc = bacc.Bacc("TRN2", target_bir_lowering=False, debug=False,
                   num_devices=cfg.ncores)
    x16s = nc.dram_tensor("x16s", [nblk * P, D], F16, kind="ExternalInput").ap()
    idx_d = nc.dram_tensor("idx", [P, total_cols], I32, kind="ExternalInput").ap()
    dl_d = nc.dram_tensor("dl", [P, total_cols], F16, kind="ExternalInput").ap()
    wv_d = nc.dram_tensor("wv", [P, total_cols], F16, kind="ExternalInput").ap()
    wl_d = nc.dram_tensor("wl", [D, D], F16, kind="ExternalInput").ap()
    wr_d = nc.dram_tensor("wr", [D, D], F16, kind="ExternalInput").ap()
    blr_d = nc.dram_tensor("blr", [1, D], F16, kind="ExternalInput").ap()
    gamma_d = nc.dram_tensor("gamma", [P, 1], F32, kind="ExternalInput").ap()
    beta_d = nc.dram_tensor("beta", [P, 1], F32, kind="ExternalInput").ap()
    iota_d = nc.dram_tensor("iota", [P, P], F16, kind="ExternalInput").ap()
    xraw_d = nc.dram_tensor("xrawT", [P, nblk * P], F16, kind="ExternalOutput").ap()
    xdesk_d = nc.dram_tensor("xdeskT", [P, nblk * P], F16, kind="ExternalOutput").ap()

    with tile.TileContext(nc) as tc:
        from contextlib import ExitStack
        with ExitStack() as ctx:
            cpool = ctx.enter_context(tc.tile_pool(name="const", bufs=1))
            gpool = ctx.enter_context(tc.tile_pool(name="gbuf", bufs=2))
            lpool = ctx.enter_context(tc.tile_pool(name="lbuf", bufs=11))
            ipool = ctx.enter_context(tc.tile_pool(name="ibuf", bufs=14))
            mpool = ctx.enter_context(tc.tile_pool(name="meta", bufs=2))
            xpool = ctx.enter_context(tc.tile_pool(name="xt", bufs=2))
            spool = ctx.enter_context(tc.tile_pool(name="stile", bufs=6))
            apool = ctx.enter_context(tc.tile_pool(name="aggT", bufs=3))
            stgp = ctx.enter_context(tc.tile_pool(name="stg", bufs=2))
            sqp = ctx.enter_context(tc.tile_pool(name="sq", bufs=2))
            ppool = ctx.enter_context(tc.tile_pool(name="parts", bufs=6))
            psA = ctx.enter_context(tc.tile_pool(name="psA", bufs=2, space="PSUM"))
            psB = ctx.enter_context(tc.tile_pool(name="psB", bufs=2, space="PSUM"))
            dpool = ctx.enter_context(tc.tile_pool(name="dram", bufs=1, space="DRAM"))

            # constants
            iota_sb = cpool.tile([P, P], F16)
            wl_sb = cpool.tile([D, D], F16)
            wr_sb = cpool.tile([D, D], F16)
            blr_sb = cpool.tile([1, D], F16)
            gamma_sb = cpool.tile([P, 1], F32)
            beta_sb = cpool.tile([P, 1], F32)
            ones_sb = cpool.tile([1, P], F16)
            sum_acc = cpool.tile([P, 1], F32)
            ssq_acc = cpool.tile([P, 1], F32)
            nc.sync.dma_start(iota_sb[:], iota_d[:])
            nc.sync.dma_start(wl_sb[:], wl_d[:])
            nc.sync.dma_start(wr_sb[:], wr_d[:])
            nc.sync.dma_start(blr_sb[:], blr_d[:])
            nc.sync.dma_start(gamma_sb[:], gamma_d[:])
            nc.sync.dma_start(beta_sb[:], beta_d[:])
            nc.vector.memset(ones_sb[:], 1.0)
            nc.vector.memset(sum_acc[:], 0.0)
            nc.vector.memset(ssq_acc[:], 0.0)

            # rebuild the full (padded) gather table on device: each core
            # uploads only its own x slice; AllGather concatenates them.
            xin = dpool.tile([nblk * P, D], F16)
            xfull = dpool.tile([cfg.ncores * nblk * P, D], F16)
            nc.gpsimd.dma_start(xin[:], x16s[:])
            nc.gpsimd.collective_compute(
                "AllGather", ALU.bypass,
                replica_groups=[list(range(cfg.ncores))],
                ins=[xin.opt()], outs=[xfull.opt()],
            )

            # precompute per-sb split geometry
            geo = []
            for si, blocks in enumerate(cfg.sblocks):
                sb_c0, sb_cols = sbinfo[si]
                lbase, rbase = {}, {}
                lc = rc = 0
                for b in blocks:
                    lbase[b], rbase[b] = lc, rc
                    lc += int(NTloc[b])
                    rc += int(NT[b]) - int(NTloc[b])
                geo.append((lbase, rbase, lc, rc))

            # prologue: local-slice gathers for all superblocks; these do not
            # depend on the x AllGather, keeping the Pool DGE stream busy
            # while the collective completes.
            idx_tiles, lbufs = {}, {}
            for si, blocks in enumerate(cfg.sblocks):
                sb_c0, sb_cols = sbinfo[si]
                lbase, rbase, lcols, rcols = geo[si]
                idx_sb = ipool.tile([P, sb_cols], I32, tag="i")
                nc.sync.dma_start(idx_sb[:], idx_d[:, sb_c0:sb_c0 + sb_cols])
                lbuf = lpool.tile([P, max(lcols, 1), P], F16, tag="l")
                for b in blocks:
                    c0 = int(colbase[b]) - sb_c0
                    for t in range(int(NTloc[b])):
                        nc.gpsimd.indirect_dma_start(
                            out=lbuf[:, lbase[b] + t, :], out_offset=None,
                            in_=x16s[:],
                            in_offset=bass.IndirectOffsetOnAxis(
                                ap=idx_sb[:, c0 + t:c0 + t + 1], axis=0),
                        )
                idx_tiles[si], lbufs[si] = idx_sb, lbuf

            for si, blocks in enumerate(cfg.sblocks):
                sb_c0, sb_cols = sbinfo[si]
                nsb = len(blocks)
                sbvalid = (nsb - 1) * P + (cfg.last_valid if blocks[-1] == nblk - 1 else P)
                lbase, rbase, lcols, rcols = geo[si]
                gbuf = gpool.tile([P, max(rcols, 1), P], F16, tag="g")
                lbuf = lbufs[si]
                idx_sb = idx_tiles[si]
                dl16 = mpool.tile([P, sb_cols], F16, tag="dl16")
                wv16 = mpool.tile([P, sb_cols], F16, tag="wv16")
                dl_sb = mpool.tile([P, sb_cols], F32, tag="dl")
                wv_sb = mpool.tile([P, sb_cols], F32, tag="wv")
                xt_sb = xpool.tile([P, nsb * P], F16, tag="xt")
                stg = stgp.tile([P, nsb * P], F16, tag="stg")

                nc.sync.dma_start(dl16[:], dl_d[:, sb_c0:sb_c0 + sb_cols])
                nc.sync.dma_start(wv16[:], wv_d[:, sb_c0:sb_c0 + sb_cols])
                nc.vector.tensor_copy(dl_sb[:], dl16[:])
                nc.vector.tensor_copy(wv_sb[:], wv16[:])
                nc.sync.dma_start_transpose(
                    xt_sb[:], x16s[blocks[0] * P:blocks[0] * P + nsb * P, :])

                for b in blocks:
                    c0 = int(colbase[b]) - sb_c0
                    for t in range(int(NTloc[b]), int(NT[b])):
                        nc.gpsimd.indirect_dma_start(
                            out=gbuf[:, rbase[b] + t - int(NTloc[b]), :],
                            out_offset=None, in_=xfull[:],
                            in_offset=bass.IndirectOffsetOnAxis(
                                ap=idx_sb[:, c0 + t:c0 + t + 1], axis=0),
                        )

                for bi, b in enumerate(blocks):
                    valid = cfg.last_valid if b == nblk - 1 else P
                    ntot = int(NT[b])
                    pa = psA.tile([P, P], F32, tag="pa", space="PSUM")
                    if ntot == 0:
                        aggT = apool.tile([P, P], F16, tag="a")
                        nc.vector.memset(aggT[:], 0.0)
                    else:
                        c0 = int(colbase[b]) - sb_c0
                        for t in range(ntot):
                            cc = c0 + t
                            st = spool.tile([P, P], F16, tag="s")
                            nc.vector.tensor_scalar(
                                st[:], iota_sb[:],
                                dl_sb[:, cc:cc + 1], wv_sb[:, cc:cc + 1],
                                ALU.is_equal, ALU.mult,
                            )
                            if t < NTloc[b]:
                                g_ap = lbuf[:, lbase[b] + t:lbase[b] + t + 1, :]
                            else:
                                rt = rbase[b] + t - int(NTloc[b])
                                g_ap = gbuf[:, rt:rt + 1, :]
                            nc.tensor.matmul(
                                out=pa[:], lhsT=g_ap, rhs=st[:],
                                start=(t == 0), stop=(t == ntot - 1),
                            )
                        aggT = apool.tile([P, P], F16, tag="a")
                        nc.scalar.activation(aggT[:], pa[:], ACT.Copy)

                    pb = psB.tile([P, P], F32, tag="pb", space="PSUM")
                    nc.tensor.matmul(out=pb[:], lhsT=wl_sb[:], rhs=aggT[:],
                                     start=True, stop=False)
                    nc.tensor.matmul(out=pb[:], lhsT=wr_sb[:],
                                     rhs=xt_sb[:, bi * P:(bi + 1) * P],
                                     start=False, stop=False)
                    nc.tensor.matmul(out=pb[:], lhsT=blr_sb[:], rhs=ones_sb[:],
                                     start=False, stop=True)

                    spart = ppool.tile([P, 1], F32, tag="sp")
                    qpart = ppool.tile([P, 1], F32, tag="qp")
                    sq = sqp.tile([P, P], F32, tag="sq")
                    nc.scalar.activation(stg[:, bi * P:bi * P + valid],
                                         pb[:, :valid], ACT.Copy, accum_out=spart[:])
                    nc.scalar.activation(sq[:, :valid], pb[:, :valid], ACT.Square,
                                         accum_out=qpart[:])
                    nc.vector.tensor_tensor(sum_acc[:], sum_acc[:], spart[:], ALU.add)
                    nc.vector.tensor_tensor(ssq_acc[:], ssq_acc[:], qpart[:], ALU.add)

                nc.sync.dma_start(xraw_d[:, blocks[0] * P:blocks[0] * P + sbvalid],
                                  stg[:, :sbvalid])

            # ---- BN stats all-reduce + scale/shift ----
            stats = cpool.tile([P, 2], F32)
            nc.vector.tensor_copy(stats[:, 0:1], sum_acc[:])
            nc.vector.tensor_copy(stats[:, 1:2], ssq_acc[:])
            cc_in = dpool.tile([P, 2], F32)
            cc_out = dpool.tile([P, 2], F32)
            nc.sync.dma_start(cc_in[:], stats[:])
            nc.gpsimd.collective_compute(
                "AllReduce", ALU.add,
                replica_groups=[list(range(cfg.ncores))],
                ins=[cc_in.opt()], outs=[cc_out.opt()],
            )
            gstats = cpool.tile([P, 2], F32)
            nc.sync.dma_start(gstats[:], cc_out[:])

            mean = cpool.tile([P, 1], F32)
            ex2 = cpool.tile([P, 1], F32)
            var = cpool.tile([P, 1], F32)
            std = cpool.tile([P, 1], F32)
            rstd = cpool.tile([P, 1], F32)
            scl = cpool.tile([P, 1], F32)
            sft = cpool.tile([P, 1], F32)
            tmp = cpool.tile([P, 1], F32)
            inv_n = 1.0 / float(N)
            nc.vector.tensor_scalar(mean[:], gstats[:, 0:1], inv_n, None, ALU.mult)
            nc.vector.tensor_scalar(ex2[:], gstats[:, 1:2], inv_n, None, ALU.mult)
            nc.vector.tensor_tensor(tmp[:], mean[:], mean[:], ALU.mult)
            nc.vector.tensor_tensor(var[:], ex2[:], tmp[:], ALU.subtract)
            nc.vector.tensor_scalar(var[:], var[:], 1e-5, None, ALU.add)
            nc.scalar.activation(std[:], var[:], ACT.Sqrt)
            nc.vector.reciprocal(rstd[:], std[:])
            nc.vector.tensor_tensor(scl[:], rstd[:], gamma_sb[:], ALU.mult)
            nc.vector.tensor_tensor(tmp[:], mean[:], scl[:], ALU.mult)
            nc.vector.tensor_tensor(sft[:], beta_sb[:], tmp[:], ALU.subtract)

            # ---- pass 2: normalize ----
            p2 = ctx.enter_context(tc.tile_pool(name="p2", bufs=2))
            for si, blocks in enumerate(cfg.sblocks):
                nsb = len(blocks)
                sbvalid = (nsb - 1) * P + (cfg.last_valid if blocks[-1] == nblk - 1 else P)
                c0 = blocks[0] * P
                xr = p2.tile([P, nsb * P], F16, tag="xr")
                xd = p2.tile([P, nsb * P], F16, tag="xd")
                nc.sync.dma_start(xr[:, :sbvalid], xraw_d[:, c0:c0 + sbvalid])
                nc.vector.tensor_scalar(xd[:, :sbvalid], xr[:, :sbvalid],
                                        scl[:], sft[:], ALU.mult, ALU.add)
                nc.sync.dma_start(xdesk_d[:, c0:c0 + sbvalid], xd[:, :sbvalid])

    nc.compile()
    return nc


_CACHE = {}


def _child_worker(conn, args):
    try:
        out = run_graph(*args, _allow_subprocess=False)
        conn.send(("ok", out))
    except BaseException as e:  # noqa: BLE001
        conn.send(("err", repr(e)))
    finally:
        conn.close()


def _run_in_subprocess(args):
    """Retry in a fresh process: a device crash can wedge the in-process
    runtime client, but a new process reconnects cleanly."""
    import multiprocessing as mp
    ctx = mp.get_context("spawn")
    parent, child = ctx.Pipe()
    p = ctx.Process(target=_child_worker, args=(child, args))
    p.start()
    status, payload = parent.recv()
    p.join()
    if status != "ok":
        raise RuntimeError(f"subprocess kernel run failed: {payload}")
    return payload


def run_graph(x, edge_index, W_l, b_l, W_r, gamma, beta, ncores=8, trace=False,
              _allow_subprocess=True):
    global LAST_EXEC_NS
    x = np.asarray(x, dtype=np.float32)
    N = x.shape[0]
    cfg = Cfg(N=N, ncores=ncores)
    NTs, per_core, shared = preprocess(cfg, x, edge_index, W_l, b_l, W_r, gamma, beta)

    key = (N, ncores, NTs[0].tobytes(), NTs[1].tobytes())
    if key not in _CACHE:
        _CACHE[key] = build_program(cfg, NTs)
    nc = _CACHE[key]

    in_maps = []
    for c in range(ncores):
        m = dict(shared)
        m.update(per_core[c])
        in_maps.append(m)

    try:
        res = run_bass_kernel_spmd(nc, in_maps, core_ids=list(range(ncores)),
                                   trace=trace)
    except Exception:
        if not _allow_subprocess:
            raise
        # transient device/runtime failure: retry in fresh processes
        args = (x, edge_index, W_l, b_l, W_r, gamma, beta, ncores, trace)
        for attempt in range(3):
            try:
                return _run_in_subprocess(args)
            except Exception:
                if attempt == 2:
                    raise
                import time as _t
                _t.sleep(15)
    LAST_EXEC_NS = res.exec_time_ns

    npc = cfg.npc
    xraw = np.empty((N, D), dtype=np.float32)
    xdesk = np.empty((N, D), dtype=np.float32)
    for c in range(ncores):
        xraw[c * npc:(c + 1) * npc] = res.results[c]["xrawT"][:, :npc].T.astype(np.float32)
        xdesk[c * npc:(c + 1) * npc] = res.results[c]["xdeskT"][:, :npc].T.astype(np.float32)
    return xraw, xdesk


def kernel(x, edge_index, W_l, b_l, W_r, gamma, beta):
    return run_graph(np.asarray(x), np.asarray(edge_index), np.asarray(W_l),
                     np.asarray(b_l), np.asarray(W_r), np.asarray(gamma),
                     np.asarray(beta), ncores=8,
                     trace=bool(int(os.environ.get("KERNEL_TRACE", "0"))))



# revision 7
# speedup vs baseline: 3.3906x; 3.3906x over previous
"""GraphSAGE layer (mean-aggr SAGEConv + BatchNorm1d) on 8 Trainium2 NeuronCores.

Strategy (edge-cut partitioning by destination node):
  - Nodes are split into 8 ranges (12500/core); each core owns all edges whose
    dst falls in its range, so aggregation completes locally. x is replicated
    to every core as a 4-row-interleaved fp16 table [25000, 512] so that
    int16 gather indices (src >> 2) cover the full node range; the low 2 bits
    of src select one of 4 column views of the table.
  - Edges are grouped by (dst block of 128, src phase = src & 3) and padded to
    128-edge tiles; per (superblock of 3 blocks, phase) a single batched
    dma_gather fetches all source rows (few large SWDGE instructions instead
    of thousands of indirect DMAs -- this removes the per-instruction
    descriptor-generation bottleneck).
  - One-hot selection matrices S[e, d] = (dl[e, tile] == d) are built on-chip
    in fp8 with a single stride-0-broadcast DVE op per superblock; PE computes
    aggsumT[f, d] += G[e, f]^T @ S[e, d] in PSUM per dst block.
  - The mean 1/deg[dst] scale is applied as one DVE multiply per superblock
    against a host-built per-column weight tile, then
    x_rawT = W_l^T @ aggT + W_r^T @ xT + b_l (PE), BatchNorm stats via the
    scalar engine's accum_out, AllReduced across cores, and a second DVE pass
    applies scale/shift in place.
"""

import os
from dataclasses import dataclass

import numpy as np

# concourse ships with the container; it is an installed package, not a sibling file.
import concourse.bacc as bacc
import concourse.bass as bass
import concourse.mybir as mybir
import concourse.tile as tile
from concourse.bass_utils import run_bass_kernel_spmd

F16 = mybir.dt.float16
F32 = mybir.dt.float32
F8 = mybir.dt.float8e3
I16 = mybir.dt.int16
ALU = mybir.AluOpType
ACT = mybir.ActivationFunctionType

D = 128
P = 128
NPH = 4  # src phases (table is 4-row interleaved to fit int16 indices)

LAST_EXEC_NS = None


@dataclass
class Cfg:
    N: int
    ncores: int = 8
    sb: int = 3  # dst blocks per superblock (gather/staging unit)

    @property
    def npc(self):
        assert self.N % self.ncores == 0
        return self.N // self.ncores

    @property
    def nblk(self):
        return (self.npc + P - 1) // P

    @property
    def npad(self):
        return self.nblk * P

    @property
    def last_valid(self):
        return self.npc - (self.nblk - 1) * P

    @property
    def sblocks(self):
        out = []
        b = 0
        while b < self.nblk:
            out.append(list(range(b, min(b + self.sb, self.nblk))))
            b += self.sb
        return out


MAX_GATHER_COLS = 8  # HW SWDGE ring limit: 1024 descriptors per dma_gather


def _layout(cfg, NT):
    """Column layout. NT[b, p] = #128-edge tiles for (dst-block b, phase p),
    shared across cores. Columns are ordered superblock -> phase -> block.
    Gather calls are emitted per (block, phase) in chunks of <=8 columns
    (<=1024 indices, the HW SWDGE ring limit); `calls` lists
    (phase, col0, ncols) with col0 global."""
    colbase = np.zeros((cfg.nblk, NPH), dtype=np.int64)
    sbinfo = []  # per sb: (c0, cols)
    calls = []  # (phase, col0, ncols) global
    col = 0
    for blocks in cfg.sblocks:
        sb_c0 = col
        for p in range(NPH):
            for b in blocks:
                colbase[b, p] = col
                nt = int(NT[b, p])
                for c0 in range(0, nt, MAX_GATHER_COLS):
                    cc = min(MAX_GATHER_COLS, nt - c0)
                    calls.append((p, col + c0, cc))
                col += nt
        sbinfo.append((sb_c0, col - sb_c0))
    return int(col), colbase, sbinfo, calls


def preprocess(cfg, x, edge_index, W_l, b_l, W_r, gamma, beta):
    N, npc, nblk = cfg.N, cfg.npc, cfg.nblk
    src = np.asarray(edge_index[0], dtype=np.int64)
    dst = np.asarray(edge_index[1], dtype=np.int64)
    E = src.shape[0]

    deg = np.bincount(dst, minlength=N)
    w_node = (1.0 / np.maximum(deg, 1.0)).astype(np.float32)

    core = dst // npc
    dloc = dst - core * npc
    blk = dloc >> 7
    din = (dloc & 127).astype(np.float16)
    ph = (src & 3).astype(np.int64)
    idxv = (src >> 2).astype(np.int16)

    # group id: (core, block, phase)
    key = (core * nblk + blk) * NPH + ph
    order = np.argsort(key, kind="stable")
    ks = key[order]
    cnt = np.bincount(key, minlength=cfg.ncores * nblk * NPH)
    cnt = cnt.reshape(cfg.ncores, nblk, NPH)
    NT = (cnt.max(axis=0) + 127) // 128  # [nblk, NPH] shared tile counts

    total_cols, colbase, sbinfo, calls = _layout(cfg, NT)
    slots = total_cols * P

    # rank of each edge within its (core, blk, phase) group
    grp_first = np.r_[0, np.flatnonzero(np.diff(ks)) + 1]
    starts = np.zeros(E, dtype=np.int64)
    starts[grp_first] = grp_first
    starts = np.maximum.accumulate(starts)
    rank = np.arange(E, dtype=np.int64) - starts

    # 4-interleaved gather table: row i = x[4i..4i+3]; shared by all cores
    xt4 = np.asarray(x, dtype=np.float16).reshape(N // 4, 4 * D)

    bounds = np.searchsorted(ks, np.arange(cfg.ncores + 1) * (nblk * NPH))
    per_core = []
    for c in range(cfg.ncores):
        a, b = bounds[c], bounds[c + 1]
        ecs = order[a:b]
        gl = ks[a:b] - c * (nblk * NPH)  # (block*NPH + phase) local group
        slot = colbase[gl // NPH, gl % NPH] * P + rank[a:b]

        a_idx = np.zeros(slots, dtype=np.int16)
        a_dl = np.full(slots, -1.0, dtype=np.float16)
        a_idx[slot] = idxv[ecs]
        a_dl[slot] = din[ecs]

        # dl: slot s -> [partition s%128, column s//128]
        dl_t = np.ascontiguousarray(a_dl.reshape(-1, P).T)
        # idx: packed per gather call: linear i = s*16 + p (p<16), replicated 8x
        idx16 = np.empty((P, total_cols * 8), dtype=np.int16)
        for p, c0, cc in calls:
            n = cc * P
            blkidx = a_idx[c0 * P:(c0 + cc) * P]
            packed = np.tile(blkidx.reshape(n // 16, 16).T, (8, 1))
            idx16[:, c0 * 8:(c0 + cc) * 8] = packed

        xTl = np.zeros((D, cfg.npad), dtype=np.float16)
        xTl[:, :npc] = np.asarray(x[c * npc:(c + 1) * npc], dtype=np.float16).T
        wbc = np.ones((P, cfg.npad), dtype=np.float16)
        wbc[:, :npc] = w_node[c * npc:(c + 1) * npc][None, :].astype(np.float16)

        per_core.append(dict(idx16=idx16, dl=dl_t, xT=xTl, wbc=wbc))

    shared = dict(
        xt4=xt4,
        wl=np.asarray(W_l, dtype=np.float16),
        wr=np.asarray(W_r, dtype=np.float16),
        blr=np.asarray(b_l, dtype=np.float16).reshape(1, D),
        gamma=np.asarray(gamma, dtype=np.float32).reshape(P, 1),
        beta=np.asarray(beta, dtype=np.float32).reshape(P, 1),
        iota=np.tile(np.arange(P, dtype=np.float16), (P, 1)),
    )
    return NT, per_core, shared


def build_program(cfg, NT):
    total_cols, colbase, sbinfo, calls = _layout(cfg, NT)
    N, nblk, npc, npad = cfg.N, cfg.nblk, cfg.npc, cfg.npad
    nquart = N // 4

    nc = bacc.Bacc("TRN2", target_bir_lowering=False, debug=False,
                   num_devices=cfg.ncores)
    xt4_d = nc.dram_tensor("xt4", [nquart, NPH * D], F16, kind="ExternalInput").ap()
    idx_d = nc.dram_tensor("idx16", [P, total_cols * 8], I16, kind="ExternalInput").ap()
    dl_d = nc.dram_tensor("dl", [P, total_cols], F16, kind="ExternalInput").ap()
    xT_d = nc.dram_tensor("xT", [D, npad], F16, kind="ExternalInput").ap()
    wbc_d = nc.dram_tensor("wbc", [P, npad], F16, kind="ExternalInput").ap()
    wl_d = nc.dram_tensor("wl", [D, D], F16, kind="ExternalInput").ap()
    wr_d = nc.dram_tensor("wr", [D, D], F16, kind="ExternalInput").ap()
    blr_d = nc.dram_tensor("blr", [1, D], F16, kind="ExternalInput").ap()
    gamma_d = nc.dram_tensor("gamma", [P, 1], F32, kind="ExternalInput").ap()
    beta_d = nc.dram_tensor("beta", [P, 1], F32, kind="ExternalInput").ap()
    iota_d = nc.dram_tensor("iota", [P, P], F16, kind="ExternalInput").ap()
    xraw_d = nc.dram_tensor("xrawT", [P, npad], F16, kind="ExternalOutput").ap()
    xdesk_d = nc.dram_tensor("xdeskT", [P, npad], F16, kind="ExternalOutput").ap()

    max_sb_cols = max(sb_cols for _, sb_cols in sbinfo)
    calls_by_sb = [[] for _ in sbinfo]
    for ci, (sb_c0, sb_cols) in enumerate(sbinfo):
        for p, c0, cc in calls:
            if sb_c0 <= c0 < sb_c0 + sb_cols:
                calls_by_sb[ci].append((p, c0, cc))

    with tile.TileContext(nc) as tc:
        from contextlib import ExitStack
        with ExitStack() as ctx:
            cpool = ctx.enter_context(tc.tile_pool(name="const", bufs=1))
            bigp = ctx.enter_context(tc.tile_pool(name="big", bufs=1))
            gpool = ctx.enter_context(tc.tile_pool(name="gbuf", bufs=2))
            spool = ctx.enter_context(tc.tile_pool(name="sbuf", bufs=2))
            ipool = ctx.enter_context(tc.tile_pool(name="ibuf", bufs=2))
            dlp = ctx.enter_context(tc.tile_pool(name="dlp", bufs=2))
            sqp = ctx.enter_context(tc.tile_pool(name="sq", bufs=2))
            ppool = ctx.enter_context(tc.tile_pool(name="parts", bufs=4))
            psA = ctx.enter_context(tc.tile_pool(name="psA", bufs=2, space="PSUM"))
            psB = ctx.enter_context(tc.tile_pool(name="psB", bufs=2, space="PSUM"))
            dpool = ctx.enter_context(tc.tile_pool(name="dram", bufs=1, space="DRAM"))

            # constants
            iota_sb = cpool.tile([P, P], F16)
            wl_sb = cpool.tile([D, D], F16)
            wr_sb = cpool.tile([D, D], F16)
            blr_sb = cpool.tile([1, D], F16)
            gamma_sb = cpool.tile([P, 1], F32)
            beta_sb = cpool.tile([P, 1], F32)
            ones_sb = cpool.tile([1, P], F16)
            sum_acc = cpool.tile([P, 1], F32)
            ssq_acc = cpool.tile([P, 1], F32)
            nc.sync.dma_start(iota_sb[:], iota_d[:])
            nc.sync.dma_start(wl_sb[:], wl_d[:])
            nc.sync.dma_start(wr_sb[:], wr_d[:])
            nc.sync.dma_start(blr_sb[:], blr_d[:])
            nc.sync.dma_start(gamma_sb[:], gamma_d[:])
            nc.sync.dma_start(beta_sb[:], beta_d[:])
            nc.vector.memset(ones_sb[:], 1.0)
            nc.vector.memset(sum_acc[:], 0.0)
            nc.vector.memset(ssq_acc[:], 0.0)

            # core-resident big tiles
            xT_sb = bigp.tile([D, npad], F16)
            wbc_sb = bigp.tile([P, npad], F16)
            aggT = bigp.tile([P, npad], F16)
            stg = bigp.tile([P, npad], F16)
            nc.sync.dma_start(xT_sb[:], xT_d[:])
            nc.sync.dma_start(wbc_sb[:], wbc_d[:])

            for si, blocks in enumerate(cfg.sblocks):
                sb_c0, sb_cols = sbinfo[si]
                nsb = len(blocks)
                if sb_cols == 0:
                    for b in blocks:
                        nc.vector.memset(aggT[:, b * P:(b + 1) * P], 0.0)
                    continue

                gbuf = gpool.tile([P, max_sb_cols, P], F16, tag="g")
                s8 = spool.tile([P, max_sb_cols, P], F8, tag="s")
                idx_sb = ipool.tile([P, max_sb_cols * 8], I16, tag="i")
                dl_sb = dlp.tile([P, max_sb_cols], F16, tag="d")

                nc.sync.dma_start(idx_sb[:, :sb_cols * 8],
                                  idx_d[:, sb_c0 * 8:(sb_c0 + sb_cols) * 8])
                nc.sync.dma_start(dl_sb[:, :sb_cols],
                                  dl_d[:, sb_c0:sb_c0 + sb_cols])

                # batched gathers (<=1024 indices per call: HW ring limit)
                for p, c0, cc in calls_by_sb[si]:
                    rel = c0 - sb_c0
                    nc.gpsimd.dma_gather(
                        out_ap=gbuf[:, rel:rel + cc, :],
                        in_ap=xt4_d[:, p * D:(p + 1) * D],
                        idxs_ap=idx_sb[:, rel * 8:(rel + cc) * 8],
                        num_idxs=cc * P, num_idxs_reg=cc * P,
                        elem_size=D, elem_step=NPH * D,
                    )

                # one-hot S for the whole superblock via stride-0 broadcasts
                dl_ap = dl_sb[:, 0:sb_cols]
                dl_b = bass.AP(dl_ap.tensor, dl_ap.offset,
                               [list(dl_ap.ap[0]), list(dl_ap.ap[1]), [0, P]])
                io_ap = iota_sb[:]
                io_b = bass.AP(io_ap.tensor, io_ap.offset,
                               [list(io_ap.ap[0]), [0, sb_cols], list(io_ap.ap[1])])
                nc.vector.tensor_tensor(s8[:, 0:sb_cols, :], dl_b, io_b,
                                        ALU.is_equal)

                # per-block aggregation: aggsumT[f, d] += G^T @ S
                for b in blocks:
                    tcols = [int(colbase[b, p]) - sb_c0 + t
                             for p in range(NPH) for t in range(int(NT[b, p]))]
                    if not tcols:
                        nc.vector.memset(aggT[:, b * P:(b + 1) * P], 0.0)
                        continue
                    pa = psA.tile([P, P], F32, tag="pa", space="PSUM")
                    for ti, cc in enumerate(tcols):
                        nc.tensor.matmul(
                            out=pa[:], lhsT=gbuf[:, cc:cc + 1, :],
                            rhs=s8[:, cc:cc + 1, :],
                            start=(ti == 0), stop=(ti == len(tcols) - 1),
                        )
                    nc.scalar.activation(aggT[:, b * P:(b + 1) * P], pa[:],
                                         ACT.Copy)

                # mean scale + SAGE linear + BN stats for this superblock
                c0 = blocks[0] * P
                npts = nsb * P
                nc.vector.tensor_tensor(aggT[:, c0:c0 + npts],
                                        aggT[:, c0:c0 + npts],
                                        wbc_sb[:, c0:c0 + npts], ALU.mult)
                for b in blocks:
                    valid = cfg.last_valid if b == nblk - 1 else P
                    pb = psB.tile([P, P], F32, tag="pb", space="PSUM")
                    nc.tensor.matmul(out=pb[:], lhsT=wl_sb[:],
                                     rhs=aggT[:, b * P:(b + 1) * P],
                                     start=True, stop=False)
                    nc.tensor.matmul(out=pb[:], lhsT=wr_sb[:],
                                     rhs=xT_sb[:, b * P:(b + 1) * P],
                                     start=False, stop=False)
                    nc.tensor.matmul(out=pb[:], lhsT=blr_sb[:], rhs=ones_sb[:],
                                     start=False, stop=True)
                    spart = ppool.tile([P, 1], F32, tag="sp")
                    qpart = ppool.tile([P, 1], F32, tag="qp")
                    sq = sqp.tile([P, P], F32, tag="sq")
                    nc.scalar.activation(stg[:, b * P:b * P + valid],
                                         pb[:, :valid], ACT.Copy,
                                         accum_out=spart[:])
                    nc.scalar.activation(sq[:, :valid], pb[:, :valid],
                                         ACT.Square, accum_out=qpart[:])
                    nc.vector.tensor_tensor(sum_acc[:], sum_acc[:], spart[:],
                                            ALU.add)
                    nc.vector.tensor_tensor(ssq_acc[:], ssq_acc[:], qpart[:],
                                            ALU.add)

            # ---- BN stats all-reduce + scale/shift ----
            stats = cpool.tile([P, 2], F32)
            nc.vector.tensor_copy(stats[:, 0:1], sum_acc[:])
            nc.vector.tensor_copy(stats[:, 1:2], ssq_acc[:])
            cc_in = dpool.tile([P, 2], F32)
            cc_out = dpool.tile([P, 2], F32)
            nc.sync.dma_start(cc_in[:], stats[:])
            nc.gpsimd.collective_compute(
                "AllReduce", ALU.add,
                replica_groups=[list(range(cfg.ncores))],
                ins=[cc_in.opt()], outs=[cc_out.opt()],
            )
            gstats = cpool.tile([P, 2], F32)
            nc.sync.dma_start(gstats[:], cc_out[:])

            mean = cpool.tile([P, 1], F32)
            ex2 = cpool.tile([P, 1], F32)
            var = cpool.tile([P, 1], F32)
            std = cpool.tile([P, 1], F32)
            rstd = cpool.tile([P, 1], F32)
            scl = cpool.tile([P, 1], F32)
            sft = cpool.tile([P, 1], F32)
            tmp = cpool.tile([P, 1], F32)
            inv_n = 1.0 / float(N)
            nc.vector.tensor_scalar(mean[:], gstats[:, 0:1], inv_n, None, ALU.mult)
            nc.vector.tensor_scalar(ex2[:], gstats[:, 1:2], inv_n, None, ALU.mult)
            nc.vector.tensor_tensor(tmp[:], mean[:], mean[:], ALU.mult)
            nc.vector.tensor_tensor(var[:], ex2[:], tmp[:], ALU.subtract)
            nc.vector.tensor_scalar(var[:], var[:], 1e-5, None, ALU.add)
            nc.scalar.activation(std[:], var[:], ACT.Sqrt)
            nc.vector.reciprocal(rstd[:], std[:])
            nc.vector.tensor_tensor(scl[:], rstd[:], gamma_sb[:], ALU.mult)
            nc.vector.tensor_tensor(tmp[:], mean[:], scl[:], ALU.mult)
            nc.vector.tensor_tensor(sft[:], beta_sb[:], tmp[:], ALU.subtract)

            # ---- outputs: xraw, then normalize stg in place -> xdesk ----
            nc.sync.dma_start(xraw_d[:], stg[:])
            nc.vector.tensor_scalar(stg[:], stg[:], scl[:], sft[:],
                                    ALU.mult, ALU.add)
            nc.sync.dma_start(xdesk_d[:], stg[:])

    nc.compile()
    return nc


_CACHE = {}


def _child_worker(conn, args):
    try:
        out = run_graph(*args, _allow_subprocess=False)
        conn.send(("ok", out))
    except BaseException as e:  # noqa: BLE001
        conn.send(("err", repr(e)))
    finally:
        conn.close()


def _run_in_subprocess(args):
    """Retry in a fresh process: a device crash can wedge the in-process
    runtime client, but a new process reconnects cleanly."""
    import multiprocessing as mp
    ctx = mp.get_context("spawn")
    parent, child = ctx.Pipe()
    p = ctx.Process(target=_child_worker, args=(child, args))
    p.start()
    status, payload = parent.recv()
    p.join()
    if status != "ok":
        raise RuntimeError(f"subprocess kernel run failed: {payload}")
    return payload


def run_graph(x, edge_index, W_l, b_l, W_r, gamma, beta, ncores=8, trace=False,
              _allow_subprocess=True):
    global LAST_EXEC_NS
    x = np.asarray(x, dtype=np.float32)
    N = x.shape[0]
    cfg = Cfg(N=N, ncores=ncores)
    NT, per_core, shared = preprocess(cfg, x, edge_index, W_l, b_l, W_r,
                                      gamma, beta)

    key = (N, ncores, NT.tobytes())
    if key not in _CACHE:
        _CACHE[key] = build_program(cfg, NT)
    nc = _CACHE[key]

    in_maps = []
    for c in range(ncores):
        m = dict(shared)
        m.update(per_core[c])
        in_maps.append(m)

    try:
        res = run_bass_kernel_spmd(nc, in_maps, core_ids=list(range(ncores)),
                                   trace=trace)
    except Exception:
        if not _allow_subprocess:
            raise
        args = (x, edge_index, W_l, b_l, W_r, gamma, beta, ncores, trace)
        for attempt in range(3):
            try:
                return _run_in_subprocess(args)
            except Exception:
                if attempt == 2:
                    raise
                import time as _t
                _t.sleep(15)
    LAST_EXEC_NS = res.exec_time_ns

    npc = cfg.npc
    xraw = np.empty((N, D), dtype=np.float32)
    xdesk = np.empty((N, D), dtype=np.float32)
    for c in range(ncores):
        xraw[c * npc:(c + 1) * npc] = res.results[c]["xrawT"][:, :npc].T.astype(np.float32)
        xdesk[c * npc:(c + 1) * npc] = res.results[c]["xdeskT"][:, :npc].T.astype(np.float32)
    return xraw, xdesk


def kernel(x, edge_index, W_l, b_l, W_r, gamma, beta):
    return run_graph(np.asarray(x), np.asarray(edge_index), np.asarray(W_l),
                     np.asarray(b_l), np.asarray(W_r), np.asarray(gamma),
                     np.asarray(beta), ncores=8,
                     trace=bool(int(os.environ.get("KERNEL_TRACE", "0"))))


# revision 11
# speedup vs baseline: 4.7561x; 1.4027x over previous
"""GraphSAGE layer (mean-aggr SAGEConv + BatchNorm1d) on 8 Trainium2 NeuronCores.

Strategy (edge-cut partitioning by destination node):
  - Nodes are split into 8 ranges (12500/core); each core owns all edges whose
    dst falls in its range, so aggregation completes locally. x is replicated
    to every core as a 4-row-interleaved fp16 table [25000, 512] so that
    int16 gather indices (src >> 2) cover the full node range; the low 2 bits
    of src select one of 4 column views of the table.
  - Edges are grouped by (dst block of 128, src phase = src & 3) and padded to
    128-edge tiles; per (superblock of 3 blocks, phase) a single batched
    dma_gather fetches all source rows (few large SWDGE instructions instead
    of thousands of indirect DMAs -- this removes the per-instruction
    descriptor-generation bottleneck).
  - One-hot selection matrices S[e, d] = (dl[e, tile] == d) are built on-chip
    in fp8 with a single stride-0-broadcast DVE op per superblock; PE computes
    aggsumT[f, d] += G[e, f]^T @ S[e, d] in PSUM per dst block.
  - The mean 1/deg[dst] scale is applied as one DVE multiply per superblock
    against a host-built per-column weight tile, then
    x_rawT = W_l^T @ aggT + W_r^T @ xT + b_l (PE), BatchNorm stats via the
    scalar engine's accum_out, AllReduced across cores, and a second DVE pass
    applies scale/shift in place.
"""

import os
from dataclasses import dataclass

import numpy as np

# concourse ships with the container; it is an installed package, not a sibling file.
import concourse.bacc as bacc
import concourse.bass as bass
import concourse.mybir as mybir
import concourse.tile as tile
from concourse.bass_utils import run_bass_kernel_spmd

F16 = mybir.dt.float16
F32 = mybir.dt.float32
F8 = mybir.dt.float8e3
I16 = mybir.dt.int16
ALU = mybir.AluOpType
ACT = mybir.ActivationFunctionType

D = 128
P = 128
NPH = 4  # src phases (table is 4-row interleaved to fit int16 indices)

LAST_EXEC_NS = None


@dataclass
class Cfg:
    N: int
    ncores: int = 8
    sb: int = 3  # dst blocks per superblock (gather/staging unit)

    @property
    def npc(self):
        assert self.N % self.ncores == 0
        return self.N // self.ncores

    @property
    def nblk(self):
        return (self.npc + P - 1) // P

    @property
    def npad(self):
        return self.nblk * P

    @property
    def last_valid(self):
        return self.npc - (self.nblk - 1) * P

    @property
    def sblocks(self):
        out = []
        b = 0
        while b < self.nblk:
            out.append(list(range(b, min(b + self.sb, self.nblk))))
            b += self.sb
        return out


MAX_GATHER_COLS = 8  # HW SWDGE ring limit: 1024 descriptors per dma_gather


def _layout(cfg, NT):
    """Column layout. NT[b, p] = #128-edge tiles for (dst-block b, phase p),
    shared across cores. Columns are ordered superblock -> phase -> block, so
    each (superblock, phase) range is contiguous and gather calls pack to the
    full 1024-index HW SWDGE ring limit across block boundaries; `calls`
    lists (phase, col0, ncols) with col0 global."""
    colbase = np.zeros((cfg.nblk, NPH), dtype=np.int64)
    sbinfo = []  # per sb: (c0, cols)
    calls = []  # (phase, col0, ncols) global
    col = 0
    for blocks in cfg.sblocks:
        sb_c0 = col
        for p in range(NPH):
            pc0 = col
            for b in blocks:
                colbase[b, p] = col
                col += int(NT[b, p])
            pcols = col - pc0
            for c0 in range(0, pcols, MAX_GATHER_COLS):
                cc = min(MAX_GATHER_COLS, pcols - c0)
                calls.append((p, pc0 + c0, cc))
        sbinfo.append((sb_c0, col - sb_c0))
    return int(col), colbase, sbinfo, calls


def preprocess(cfg, x, edge_index, W_l, b_l, W_r, gamma, beta):
    N, npc, nblk = cfg.N, cfg.npc, cfg.nblk
    src = np.asarray(edge_index[0], dtype=np.int64)
    dst = np.asarray(edge_index[1], dtype=np.int64)
    E = src.shape[0]

    deg = np.bincount(dst, minlength=N)
    w_node = (1.0 / np.maximum(deg, 1.0)).astype(np.float32)

    core = dst // npc
    dloc = dst - core * npc
    blk = dloc >> 7
    din = (dloc & 127).astype(np.float16)
    ph = (src & 3).astype(np.int64)
    idxv = (src >> 2).astype(np.int16)

    # group id: (core, block, phase)
    key = (core * nblk + blk) * NPH + ph
    order = np.argsort(key, kind="stable")
    ks = key[order]
    cnt = np.bincount(key, minlength=cfg.ncores * nblk * NPH)
    cnt = cnt.reshape(cfg.ncores, nblk, NPH)
    NT = (cnt.max(axis=0) + 127) // 128  # [nblk, NPH] shared tile counts

    total_cols, colbase, sbinfo, calls = _layout(cfg, NT)
    slots = total_cols * P

    # rank of each edge within its (core, blk, phase) group
    grp_first = np.r_[0, np.flatnonzero(np.diff(ks)) + 1]
    starts = np.zeros(E, dtype=np.int64)
    starts[grp_first] = grp_first
    starts = np.maximum.accumulate(starts)
    rank = np.arange(E, dtype=np.int64) - starts

    # 4-interleaved gather table: row i = x[4i..4i+3]; shared by all cores
    xt4 = np.asarray(x, dtype=np.float16).reshape(N // 4, 4 * D)

    bounds = np.searchsorted(ks, np.arange(cfg.ncores + 1) * (nblk * NPH))
    per_core = []
    for c in range(cfg.ncores):
        a, b = bounds[c], bounds[c + 1]
        ecs = order[a:b]
        gl = ks[a:b] - c * (nblk * NPH)  # (block*NPH + phase) local group
        slot = colbase[gl // NPH, gl % NPH] * P + rank[a:b]

        a_idx = np.zeros(slots, dtype=np.int16)
        a_dl = np.full(slots, -1.0, dtype=np.float16)
        a_idx[slot] = idxv[ecs]
        a_dl[slot] = din[ecs]

        # dl: slot s -> [partition s%128, column s//128]
        dl_t = np.ascontiguousarray(a_dl.reshape(-1, P).T)
        # idx: packed per gather call: linear i = s*16 + p (p<16), replicated 8x
        idx16 = np.empty((P, total_cols * 8), dtype=np.int16)
        for p, c0, cc in calls:
            n = cc * P
            blkidx = a_idx[c0 * P:(c0 + cc) * P]
            packed = np.tile(blkidx.reshape(n // 16, 16).T, (8, 1))
            idx16[:, c0 * 8:(c0 + cc) * 8] = packed

        xTl = np.zeros((D, cfg.npad), dtype=np.float16)
        xTl[:, :npc] = np.asarray(x[c * npc:(c + 1) * npc], dtype=np.float16).T
        wbc = np.ones((P, cfg.npad), dtype=np.float16)
        wbc[:, :npc] = w_node[c * npc:(c + 1) * npc][None, :].astype(np.float16)

        per_core.append(dict(idx16=idx16, dl=dl_t, xT=xTl, wbc=wbc))

    shared = dict(
        xt4=xt4,
        wl=np.asarray(W_l, dtype=np.float16),
        wr=np.asarray(W_r, dtype=np.float16),
        blr=np.asarray(b_l, dtype=np.float16).reshape(1, D),
        gamma=np.asarray(gamma, dtype=np.float32).reshape(P, 1),
        beta=np.asarray(beta, dtype=np.float32).reshape(P, 1),
        iota=np.tile(np.arange(P, dtype=np.float16), (P, 1)),
    )
    return NT, per_core, shared


def build_program(cfg, NT, skip_compute=False, gbufs=2):
    total_cols, colbase, sbinfo, calls = _layout(cfg, NT)
    N, nblk, npc, npad = cfg.N, cfg.nblk, cfg.npc, cfg.npad
    nquart = N // 4

    nc = bacc.Bacc("TRN2", target_bir_lowering=False, debug=False,
                   num_devices=cfg.ncores)
    xt4_d = nc.dram_tensor("xt4", [nquart, NPH * D], F16, kind="ExternalInput").ap()
    idx_d = nc.dram_tensor("idx16", [P, total_cols * 8], I16, kind="ExternalInput").ap()
    dl_d = nc.dram_tensor("dl", [P, total_cols], F16, kind="ExternalInput").ap()
    xT_d = nc.dram_tensor("xT", [D, npad], F16, kind="ExternalInput").ap()
    wbc_d = nc.dram_tensor("wbc", [P, npad], F16, kind="ExternalInput").ap()
    wl_d = nc.dram_tensor("wl", [D, D], F16, kind="ExternalInput").ap()
    wr_d = nc.dram_tensor("wr", [D, D], F16, kind="ExternalInput").ap()
    blr_d = nc.dram_tensor("blr", [1, D], F16, kind="ExternalInput").ap()
    gamma_d = nc.dram_tensor("gamma", [P, 1], F32, kind="ExternalInput").ap()
    beta_d = nc.dram_tensor("beta", [P, 1], F32, kind="ExternalInput").ap()
    iota_d = nc.dram_tensor("iota", [P, P], F16, kind="ExternalInput").ap()
    xraw_d = nc.dram_tensor("xrawT", [P, npad], F16, kind="ExternalOutput").ap()
    xdesk_d = nc.dram_tensor("xdeskT", [P, npad], F16, kind="ExternalOutput").ap()

    max_sb_cols = max(sb_cols for _, sb_cols in sbinfo)
    calls_by_sb = [[] for _ in sbinfo]
    for ci, (sb_c0, sb_cols) in enumerate(sbinfo):
        for p, c0, cc in calls:
            if sb_c0 <= c0 < sb_c0 + sb_cols:
                calls_by_sb[ci].append((p, c0, cc))

    with tile.TileContext(nc) as tc:
        from contextlib import ExitStack
        with ExitStack() as ctx:
            cpool = ctx.enter_context(tc.tile_pool(name="const", bufs=1))
            bigp = ctx.enter_context(tc.tile_pool(name="big", bufs=1))
            gpool = ctx.enter_context(tc.tile_pool(name="gbuf", bufs=gbufs))
            spool = ctx.enter_context(tc.tile_pool(name="sbuf", bufs=gbufs))
            ipool = ctx.enter_context(tc.tile_pool(name="ibuf", bufs=gbufs))
            dlp = ctx.enter_context(tc.tile_pool(name="dlp", bufs=gbufs))
            sqp = ctx.enter_context(tc.tile_pool(name="sq", bufs=2))
            ppool = ctx.enter_context(tc.tile_pool(name="parts", bufs=4))
            psA = ctx.enter_context(tc.tile_pool(name="psA", bufs=2, space="PSUM"))
            psB = ctx.enter_context(tc.tile_pool(name="psB", bufs=2, space="PSUM"))
            dpool = ctx.enter_context(tc.tile_pool(name="dram", bufs=1, space="DRAM"))

            # constants
            iota_sb = cpool.tile([P, P], F16)
            wl_sb = cpool.tile([D, D], F16)
            wr_sb = cpool.tile([D, D], F16)
            blr_sb = cpool.tile([1, D], F16)
            gamma_sb = cpool.tile([P, 1], F32)
            beta_sb = cpool.tile([P, 1], F32)
            ones_sb = cpool.tile([1, P], F16)
            sum_acc = cpool.tile([P, 1], F32)
            ssq_acc = cpool.tile([P, 1], F32)
            nc.sync.dma_start(iota_sb[:], iota_d[:])
            nc.sync.dma_start(wl_sb[:], wl_d[:])
            nc.sync.dma_start(wr_sb[:], wr_d[:])
            nc.sync.dma_start(blr_sb[:], blr_d[:])
            nc.sync.dma_start(gamma_sb[:], gamma_d[:])
            nc.sync.dma_start(beta_sb[:], beta_d[:])
            nc.vector.memset(ones_sb[:], 1.0)
            nc.vector.memset(sum_acc[:], 0.0)
            nc.vector.memset(ssq_acc[:], 0.0)

            # core-resident big tiles
            xT_sb = bigp.tile([D, npad], F16)
            wbc_sb = bigp.tile([P, npad], F16)
            aggT = bigp.tile([P, npad], F16)
            stg = bigp.tile([P, npad], F16)
            nc.sync.dma_start(xT_sb[:], xT_d[:])
            nc.sync.dma_start(wbc_sb[:], wbc_d[:])

            for si, blocks in enumerate(cfg.sblocks):
                sb_c0, sb_cols = sbinfo[si]
                nsb = len(blocks)
                if sb_cols == 0:
                    for b in blocks:
                        nc.vector.memset(aggT[:, b * P:(b + 1) * P], 0.0)
                    continue

                gbuf = gpool.tile([P, max_sb_cols, P], F16, tag="g")
                s8 = spool.tile([P, max_sb_cols, P], F8, tag="s")
                idx_sb = ipool.tile([P, max_sb_cols * 8], I16, tag="i")
                dl_sb = dlp.tile([P, max_sb_cols], F16, tag="d")

                nc.sync.dma_start(idx_sb[:, :sb_cols * 8],
                                  idx_d[:, sb_c0 * 8:(sb_c0 + sb_cols) * 8])
                nc.sync.dma_start(dl_sb[:, :sb_cols],
                                  dl_d[:, sb_c0:sb_c0 + sb_cols])

                # batched gathers (<=1024 indices per call: HW ring limit)
                for p, c0, cc in calls_by_sb[si]:
                    rel = c0 - sb_c0
                    nc.gpsimd.dma_gather(
                        out_ap=gbuf[:, rel:rel + cc, :],
                        in_ap=xt4_d[:, p * D:(p + 1) * D],
                        idxs_ap=idx_sb[:, rel * 8:(rel + cc) * 8],
                        num_idxs=cc * P, num_idxs_reg=cc * P,
                        elem_size=D, elem_step=NPH * D,
                    )

                if skip_compute:
                    # ablation: touch the gathered tile so the ring recycles
                    nc.vector.tensor_copy(aggT[:, blocks[0] * P:blocks[0] * P + P],
                                          gbuf[:, 0, :])
                    continue

                # one-hot S for the whole superblock via stride-0 broadcasts
                dl_ap = dl_sb[:, 0:sb_cols]
                dl_b = bass.AP(dl_ap.tensor, dl_ap.offset,
                               [list(dl_ap.ap[0]), list(dl_ap.ap[1]), [0, P]])
                io_ap = iota_sb[:]
                io_b = bass.AP(io_ap.tensor, io_ap.offset,
                               [list(io_ap.ap[0]), [0, sb_cols], list(io_ap.ap[1])])
                nc.vector.tensor_tensor(s8[:, 0:sb_cols, :], dl_b, io_b,
                                        ALU.is_equal)

                # per-block aggregation: aggsumT[f, d] += G^T @ S
                for b in blocks:
                    tcols = [int(colbase[b, p]) - sb_c0 + t
                             for p in range(NPH) for t in range(int(NT[b, p]))]
                    if not tcols:
                        nc.vector.memset(aggT[:, b * P:(b + 1) * P], 0.0)
                        continue
                    pa = psA.tile([P, P], F32, tag="pa", space="PSUM")
                    for ti, cc in enumerate(tcols):
                        nc.tensor.matmul(
                            out=pa[:], lhsT=gbuf[:, cc:cc + 1, :],
                            rhs=s8[:, cc:cc + 1, :],
                            start=(ti == 0), stop=(ti == len(tcols) - 1),
                        )
                    nc.scalar.activation(aggT[:, b * P:(b + 1) * P], pa[:],
                                         ACT.Copy)

                # mean scale + SAGE linear + BN stats for this superblock
                c0 = blocks[0] * P
                npts = nsb * P
                nc.vector.tensor_tensor(aggT[:, c0:c0 + npts],
                                        aggT[:, c0:c0 + npts],
                                        wbc_sb[:, c0:c0 + npts], ALU.mult)
                for b in blocks:
                    valid = cfg.last_valid if b == nblk - 1 else P
                    pb = psB.tile([P, P], F32, tag="pb", space="PSUM")
                    nc.tensor.matmul(out=pb[:], lhsT=wl_sb[:],
                                     rhs=aggT[:, b * P:(b + 1) * P],
                                     start=True, stop=False)
                    nc.tensor.matmul(out=pb[:], lhsT=wr_sb[:],
                                     rhs=xT_sb[:, b * P:(b + 1) * P],
                                     start=False, stop=False)
                    nc.tensor.matmul(out=pb[:], lhsT=blr_sb[:], rhs=ones_sb[:],
                                     start=False, stop=True)
                    spart = ppool.tile([P, 1], F32, tag="sp")
                    qpart = ppool.tile([P, 1], F32, tag="qp")
                    sq = sqp.tile([P, P], F32, tag="sq")
                    nc.scalar.activation(stg[:, b * P:b * P + valid],
                                         pb[:, :valid], ACT.Copy,
                                         accum_out=spart[:])
                    nc.scalar.activation(sq[:, :valid], pb[:, :valid],
                                         ACT.Square, accum_out=qpart[:])
                    nc.vector.tensor_tensor(sum_acc[:], sum_acc[:], spart[:],
                                            ALU.add)
                    nc.vector.tensor_tensor(ssq_acc[:], ssq_acc[:], qpart[:],
                                            ALU.add)

            # ---- BN stats all-reduce + scale/shift ----
            stats = cpool.tile([P, 2], F32)
            nc.vector.tensor_copy(stats[:, 0:1], sum_acc[:])
            nc.vector.tensor_copy(stats[:, 1:2], ssq_acc[:])
            cc_in = dpool.tile([P, 2], F32)
            cc_out = dpool.tile([P, 2], F32)
            nc.sync.dma_start(cc_in[:], stats[:])
            nc.gpsimd.collective_compute(
                "AllReduce", ALU.add,
                replica_groups=[list(range(cfg.ncores))],
                ins=[cc_in.opt()], outs=[cc_out.opt()],
            )
            gstats = cpool.tile([P, 2], F32)
            nc.sync.dma_start(gstats[:], cc_out[:])

            mean = cpool.tile([P, 1], F32)
            ex2 = cpool.tile([P, 1], F32)
            var = cpool.tile([P, 1], F32)
            std = cpool.tile([P, 1], F32)
            rstd = cpool.tile([P, 1], F32)
            scl = cpool.tile([P, 1], F32)
            sft = cpool.tile([P, 1], F32)
            tmp = cpool.tile([P, 1], F32)
            inv_n = 1.0 / float(N)
            nc.vector.tensor_scalar(mean[:], gstats[:, 0:1], inv_n, None, ALU.mult)
            nc.vector.tensor_scalar(ex2[:], gstats[:, 1:2], inv_n, None, ALU.mult)
            nc.vector.tensor_tensor(tmp[:], mean[:], mean[:], ALU.mult)
            nc.vector.tensor_tensor(var[:], ex2[:], tmp[:], ALU.subtract)
            nc.vector.tensor_scalar(var[:], var[:], 1e-5, None, ALU.add)
            nc.scalar.activation(std[:], var[:], ACT.Sqrt)
            nc.vector.reciprocal(rstd[:], std[:])
            nc.vector.tensor_tensor(scl[:], rstd[:], gamma_sb[:], ALU.mult)
            nc.vector.tensor_tensor(tmp[:], mean[:], scl[:], ALU.mult)
            nc.vector.tensor_tensor(sft[:], beta_sb[:], tmp[:], ALU.subtract)

            # ---- outputs: xraw, then normalize stg in place -> xdesk ----
            nc.sync.dma_start(xraw_d[:], stg[:])
            nc.vector.tensor_scalar(stg[:], stg[:], scl[:], sft[:],
                                    ALU.mult, ALU.add)
            nc.sync.dma_start(xdesk_d[:], stg[:])

    nc.compile()
    return nc


_CACHE = {}


def _child_worker(conn, args):
    try:
        out = run_graph(*args, _allow_subprocess=False)
        conn.send(("ok", out))
    except BaseException as e:  # noqa: BLE001
        conn.send(("err", repr(e)))
    finally:
        conn.close()


def _run_in_subprocess(args):
    """Retry in a fresh process: a device crash can wedge the in-process
    runtime client, but a new process reconnects cleanly."""
    import multiprocessing as mp
    ctx = mp.get_context("spawn")
    parent, child = ctx.Pipe()
    p = ctx.Process(target=_child_worker, args=(child, args))
    p.start()
    status, payload = parent.recv()
    p.join()
    if status != "ok":
        raise RuntimeError(f"subprocess kernel run failed: {payload}")
    return payload


def run_graph(x, edge_index, W_l, b_l, W_r, gamma, beta, ncores=8, trace=False,
              _allow_subprocess=True):
    global LAST_EXEC_NS
    x = np.asarray(x, dtype=np.float32)
    N = x.shape[0]
    cfg = Cfg(N=N, ncores=ncores)
    NT, per_core, shared = preprocess(cfg, x, edge_index, W_l, b_l, W_r,
                                      gamma, beta)

    key = (N, ncores, NT.tobytes())
    if key not in _CACHE:
        _CACHE[key] = build_program(cfg, NT)
    nc = _CACHE[key]

    in_maps = []
    for c in range(ncores):
        m = dict(shared)
        m.update(per_core[c])
        in_maps.append(m)

    try:
        res = run_bass_kernel_spmd(nc, in_maps, core_ids=list(range(ncores)),
                                   trace=trace)
    except Exception:
        if not _allow_subprocess:
            raise
        args = (x, edge_index, W_l, b_l, W_r, gamma, beta, ncores, trace)
        for attempt in range(3):
            try:
                return _run_in_subprocess(args)
            except Exception:
                if attempt == 2:
                    raise
                import time as _t
                _t.sleep(15)
    LAST_EXEC_NS = res.exec_time_ns

    npc = cfg.npc
    xraw = np.empty((N, D), dtype=np.float32)
    xdesk = np.empty((N, D), dtype=np.float32)
    for c in range(ncores):
        xraw[c * npc:(c + 1) * npc] = res.results[c]["xrawT"][:, :npc].T.astype(np.float32)
        xdesk[c * npc:(c + 1) * npc] = res.results[c]["xdeskT"][:, :npc].T.astype(np.float32)
    return xraw, xdesk


def kernel(x, edge_index, W_l, b_l, W_r, gamma, beta):
    return run_graph(np.asarray(x), np.asarray(edge_index), np.asarray(W_l),
                     np.asarray(b_l), np.asarray(W_r), np.asarray(gamma),
                     np.asarray(beta), ncores=8,
                     trace=bool(int(os.environ.get("KERNEL_TRACE", "0"))))
